# revision 23
# baseline (speedup 1.0000x reference)
"""Trainium2 Bass kernel for nn_Attention_28724741275862.

Reference computation (per batch b):
    dec_part[i,o] = dec[b] @ W_dec.T          # [64, 512]
    enc_part[j,o] = enc[b] @ W_enc.T          # [512, 512]
    logits[i,j,o] = dec_part[i,o] + enc_part[j,o] + bias[o]
    alpha = log_softmax(logits, axis=o)
    ctx[i,o] = sum_j alpha[i,j,o] * enc[b][j,o]

Factorization (exact in fp32; ~7e-3 max-rel with fp8 operands):
    LSE[i,j] = ln sum_o exp(dec_part[i,o]) * exp(enc_part[j,o] + b[o])
    LSE'     = LSE - MU   (MU hardcoded; implemented as ln(S_mat * e^-MU))
    ctx[i,o] = dec_part[i,o]*S[o] + fix[o] - sum_j LSE'[i,j]*enc[j,o]
      S[o]   = sum_j enc[j,o]
      fix[o] = (b[o]-MU)*S[o] + sum_j enc_part[j,o]*enc[j,o]

Pipeline (per core = per batch; transposed layout: features on partitions):
    A1 (PE):  enc_part^T = W_enc^T.T @ enc^T — fp8 DoubleRow (2 d-blocks per
              pass), ob-staggered stops so the exps pipeline behind it
    ee (ACT): exp(enc_part^T + b) per ob as each pp[ob] completes; fp8 out
    A2 (PE):  dec_part^T (fp8), slotted between A1 ob1 and ob2 so that
              ed (ACT) = exp(dec_part^T) lands before ee2/ee3 finish
    C  (PE):  S_mat^T[j,i] += ee[ob].T @ ed[ob] per ob as exps complete
    Ln (ACT): LSE'^T = ln(S_mat^T * e^-MU), fp8
    E  (PE):  pc = enc[j,o].T @ LSE'^T (fp8), then DVE: ctx^T = ctmp - pc
    DVE:      S via reduce_sum(enc^T); fix via one fused stts per ob;
              ctmp = dec_part^T*S + fix; final subs; out DMA in 2 halves

DMA: 4 symmetric waves per HWDGE ring (SP=partitions 0:64, ACT=64:128):
    esb (enc^T fp8) -> wen (W_enc^T fp8) -> bt (W_dec^T|dec^T fp8) -> eNs
    (enc[j,o] fp8).  Same wave on both rings => no cross-wave interleaving;
    per-ring FIFO keeps wave k ahead of wave k+1.  b4 rides a tiny 5th DMA
    on SP.  All inputs fp8 => 1.06 MB/core (vs 1.64 MB baseline).
    PE warm-up junk matmuls start right at block entry (jbf memset happens
    on GPSIMD in the preamble) so the HAM clock-gate lifts ~3.4us later.

Sharding: data-parallel over batch B=8 across 8 cores; no collectives.
Self-contained: hardcodes B=8, T_dec=64, T_enc=512, H2=512.
"""

import sys

for _p in ("/opt/trn_rl_repo",):
    if _p not in sys.path:
        sys.path.insert(0, _p)

import numpy as np
import ml_dtypes

import concourse.bass as bass
from concourse import bacc, mybir
from concourse.bass_utils import run_bass_kernel_spmd

B, T_DEC, T_ENC, H2 = 8, 64, 512, 512
P = 128  # SBUF partitions
NB = H2 // P  # 4 feature blocks
MU = 7.0  # LSE shift; ln(S*e^-MU) keeps LSE' in fp8 sweet spot
N_WARMUP = 10  # PE HAM warmup matmuls (256-wide junk)
USE_DR = True  # fp8 DoubleRow for A1
PREAMBLE_MEMSET = True  # jbf memset on GPSIMD in preamble vs DVE in block
FP8_INTER = True  # ee/ed/lt/eNs in fp8 (else bf16)

BF16 = mybir.dt.bfloat16
F32 = mybir.dt.float32

_CACHE = {}

from contextlib import ExitStack


def build_raw(bacc, mybir, bass):
    BF16 = mybir.dt.bfloat16
    F32 = mybir.dt.float32
    FP8 = mybir.dt.float8e4
    AF = mybir.ActivationFunctionType
    ALU = mybir.AluOpType
    DR = mybir.MatmulPerfMode.DoubleRow
    SCALE = float(np.exp(-MU))

    nc = bacc.Bacc(None, target_bir_lowering=False)

    # DRAM inputs, p-major: every DMA is one long contiguous run per partition.
    esbd = nc.dram_tensor("esb", [P, NB, 256], BF16, kind="ExternalInput")
    wend = nc.dram_tensor("wen", [P, NB, NB, 64], BF16, kind="ExternalInput")
    b4d = nc.dram_tensor("b4", [P, 8], F32, kind="ExternalInput")
    btd = nc.dram_tensor("bt", [P, 1152], BF16, kind="ExternalInput")
    eNd = nc.dram_tensor("eN", [P, NB, 256 if FP8_INTER else 512], BF16, kind="ExternalInput")
    outd = nc.dram_tensor("out", [P, NB, T_DEC], F32, kind="ExternalOutput")

    with ExitStack() as ctx:
        ec = ctx.enter_context
        # ---- SBUF ----
        esb = ec(nc.sbuf_tensor("esbs", [P, NB, 256], BF16))  # enc^T fp8
        wen = ec(nc.sbuf_tensor("wens", [P, NB, NB, 64], BF16))  # W_enc^T fp8
        b4s = ec(nc.sbuf_tensor("b4s", [P, 8], F32))  # b4 | b4m
        bt = ec(nc.sbuf_tensor("bts", [P, 1152], BF16))  # W_dec^T fp8 | dec^T fp8
        IW = 1 if FP8_INTER else 2  # slot width multiplier for intermediates
        eNs = ec(nc.sbuf_tensor("eNs", [P, NB, 256 * IW], BF16))  # enc[j,o]
        ee = ec(nc.sbuf_tensor("ee", [P, NB, 256 * IW], BF16))  # exp(enc_part^T+b)
        ed = ec(nc.sbuf_tensor("ed", [P, NB, 32 * IW], BF16))  # exp(dec_part^T)
        lt = ec(nc.sbuf_tensor("lt", [P, NB, 32 * IW], BF16))  # LSE'^T
        se = ec(nc.sbuf_tensor("se", [P, NB], F32))  # S_enc
        cp = ec(nc.sbuf_tensor("cp", [P, NB], F32))  # fix
        junkD = ec(nc.sbuf_tensor("junkD", [P, NB, 256], BF16))  # stts sink
        ctxo = ec(nc.sbuf_tensor("ctxo", [P, NB, T_DEC], F32))
        jbf = ec(nc.sbuf_tensor("jbf", [P, 256], BF16))  # PE warmup junk
        wj = ec(nc.sbuf_tensor("wj", [P, NB], F32))  # ACT warmup junk
        # ---- PSUM (8 banks exactly) ----
        pp = [ec(nc.psum_tensor(f"pp{o}", [P, T_ENC], F32)) for o in range(NB)]
        pd = ec(nc.psum_tensor("pd", [P, NB, T_DEC], F32))
        # C output S^T: two banks so two jb accumulation groups can be open
        # at once (one group per PSUM zero-region). Slot order (jb0,jb2|jb1,jb3).
        psX = ec(nc.psum_tensor("psX", [P, 2, T_DEC], F32))
        psY = ec(nc.psum_tensor("psY", [P, 2, T_DEC], F32))
        pcb = ec(nc.psum_tensor("pcb", [P, NB, T_DEC], F32))  # E output

        # C bank/slot for jb; lt slot order is (jb0, jb2, jb1, jb3)
        C_DST = {0: (0, 0), 1: (1, 0), 2: (0, 1), 3: (1, 1)}  # jb -> (bankY?, slot)
        LT_SLOT = {0: 0, 2: 1, 1: 2, 3: 3}

        b4ap = b4s[:, 0:4]  # [P, 4] true bias
        b4m = b4s[:, 4:8]  # [P, 4] bias - MU

        def wen_dr(ob, pair):  # A1 lhsT: [128 d, 2 kblk, 128 o] fp8
            return wen[:, 2 * pair : 2 * pair + 2, ob, :].bitcast(FP8)

        def esb_dr(pair):  # A1 rhs: [128 d, 2 kblk, 512 j] fp8
            return esb[:, 2 * pair : 2 * pair + 2, :].bitcast(FP8)

        def i8(ap):  # intermediates: fp8 view or plain bf16
            return ap.bitcast(FP8) if FP8_INTER else ap

        IB = 64 if FP8_INTER else 128  # intermediate slot block (128 elems)

        def wen_k(ob, k):  # plain A1 lhsT: [128 d, 128 o] fp8
            return wen[:, k, ob, :].bitcast(FP8)

        def esb_k(k):  # plain A1 rhs: [128 d, 512 j] fp8
            return esb[:, k, :].bitcast(FP8)

        def wdec_s(k, ob):  # A2 lhsT: [128 d, 128 o] fp8
            off = k * 256 + ob * 64
            return bt[:, off : off + 64].bitcast(FP8)

        def dect_s(k):  # A2 rhs: [128 d, 64 i] fp8
            off = 1024 + k * 32
            return bt[:, off : off + 32].bitcast(FP8)

        # ---- semaphores ----
        jz = ec(nc.semaphore("jz"))
        dE = ec(nc.semaphore("dE"))
        dW = ec(nc.semaphore("dW"))
        dB = ec(nc.semaphore("dB"))
        dA = ec(nc.semaphore("dA"))
        dN = ec(nc.semaphore("dN"))
        pe = ec(nc.semaphore("pe"))  # PE stops: A1 1,2 A2 3-6 A1 7,8 C 9-12 E 13-16
        ac = ec(nc.semaphore("ac"))  # ACT: warm 1, ee0 2, ee1 3, ed 4, ee2 5, ee3 6, ln 7
        dv = ec(nc.semaphore("dv"))  # DVE progress
        dO = ec(nc.semaphore("dO"))  # out DMAs (2 x +16)

        # Preamble (runs in main before the block): jbf ready for PE warmup.
        if PREAMBLE_MEMSET:
            nc.gpsimd.memset(jbf[:, :], 0.0).then_inc(jz, 1)

        with nc.Block(no_gpsimd_drain=True) as block:

            @block.sync
            def _(sync):
                sync.dma_start(out=esb[0:64], in_=esbd[0:64]).then_inc(dE, 16)
                sync.dma_start(out=wen[0:64], in_=wend[0:64]).then_inc(dW, 16)
                sync.dma_start(out=b4s[:, :], in_=b4d[:, :]).then_inc(dB, 16)
                sync.dma_start(out=bt[0:64], in_=btd[0:64]).then_inc(dA, 16)
                sync.dma_start(out=eNs[0:64], in_=eNd[0:64]).then_inc(dN, 16)
                sync.wait_ge(dv, 10)
                sync.dma_start(out=outd[:, 0:2, :], in_=ctxo[:, 0:2, :]).then_inc(
                    dO, 16
                )
                sync.wait_ge(dv, 11)
                sync.dma_start(out=outd[:, 2:NB, :], in_=ctxo[:, 2:NB, :]).then_inc(
                    dO, 16
                )
                sync.wait_ge(dO, 32)

            @block.scalar
            def _(scalar):
                # Exp table load hoists to the top of this block (async).
                scalar.activation(wj[:, 0:1], wj[:, 3:4], AF.Exp, scale=0.0).then_inc(
                    ac, 1
                )  # ac=1
                scalar.dma_start(out=esb[64:P], in_=esbd[64:P]).then_inc(dE, 16)
                scalar.dma_start(out=wen[64:P], in_=wend[64:P]).then_inc(dW, 16)
                scalar.dma_start(out=bt[64:P], in_=btd[64:P]).then_inc(dA, 16)
                scalar.dma_start(out=eNs[64:P], in_=eNd[64:P]).then_inc(dN, 16)
                scalar.wait_ge(dB, 16)
                # ee[ob] = exp(enc_part^T + b), fp8 out, as pp[ob] completes
                scalar.wait_ge(pe, 1)
                scalar.activation(
                    i8(ee[:, 0, :]), pp[0][:, :], AF.Exp,
                    bias=b4ap[:, 0:1],
                ).then_inc(ac, 1)  # ac=2
                scalar.wait_ge(pe, 2)
                scalar.activation(
                    i8(ee[:, 1, :]), pp[1][:, :], AF.Exp,
                    bias=b4ap[:, 1:2],
                ).then_inc(ac, 1)  # ac=3
                # ed = exp(dec_part^T) right after A2
                scalar.wait_ge(pe, 6)
                scalar.activation(
                    i8(ed[:, :, :]), pd[:, :, :], AF.Exp
                ).then_inc(ac, 1)  # ac=4
                scalar.wait_ge(pe, 7)
                scalar.activation(
                    i8(ee[:, 2, :]), pp[2][:, :], AF.Exp,
                    bias=b4ap[:, 2:3],
                ).then_inc(ac, 1)  # ac=5
                scalar.wait_ge(pe, 8)
                scalar.activation(
                    i8(ee[:, 3, :]), pp[3][:, :], AF.Exp,
                    bias=b4ap[:, 3:4],
                ).then_inc(ac, 1)  # ac=6
                # Ln table pull-forward
                scalar.activation(wj[:, 1:2], wj[:, 3:4], AF.Ln, bias=1.0, scale=0.0)
                # LSE'^T = ln(S^T * e^-MU), fp8, split per C bank to pipeline
                scalar.wait_ge(pe, 11)
                scalar.activation(
                    i8(lt[:, 0:2, :]), psX[:, :, :], AF.Ln, scale=SCALE
                ).then_inc(ac, 1)  # ac=7
                scalar.wait_ge(pe, 12)
                scalar.activation(
                    i8(lt[:, 2:NB, :]), psY[:, :, :], AF.Ln, scale=SCALE
                ).then_inc(ac, 1)  # ac=8

            @block.tensor
            def _(tensor):
                # HAM warmup junk matmuls into the (later overwritten) pd bank
                tensor.wait_ge(jz, 1)
                for _ in range(N_WARMUP):
                    tensor.matmul(
                        pd[:, :, :],
                        lhsT=jbf[:, 0:P],
                        rhs=jbf[:, :],
                        start=True,
                        stop=True,
                    )
                # A1 ob0, ob1 (fp8 DoubleRow: k-pairs (0,1) and (2,3))
                tensor.wait_ge(dE, 32)
                tensor.wait_ge(dW, 32)
                for ob in (0, 1):
                    if USE_DR:
                        tensor.matmul(
                            pp[ob][:, :], lhsT=wen_dr(ob, 0), rhs=esb_dr(0),
                            start=True, stop=False, perf_mode=DR,
                        )
                        mm = tensor.matmul(
                            pp[ob][:, :], lhsT=wen_dr(ob, 1), rhs=esb_dr(1),
                            start=False, stop=True, perf_mode=DR,
                        )
                    else:
                        for k in range(NB):
                            mm = tensor.matmul(
                                pp[ob][:, :], lhsT=wen_k(ob, k), rhs=esb_k(k),
                                start=(k == 0), stop=(k == NB - 1),
                            )
                    mm.then_inc(pe, 1)  # pe 1,2
                # A2: dec_part^T (fp8), so ed can run early on ACT
                tensor.wait_ge(dA, 32)
                for ob in range(NB):
                    for k in range(NB):
                        mm = tensor.matmul(
                            pd[:, ob, :], lhsT=wdec_s(k, ob), rhs=dect_s(k),
                            start=(k == 0), stop=(k == NB - 1),
                        )
                        if k == NB - 1:
                            mm.then_inc(pe, 1)  # pe 3..6
                # A1 ob2, ob3
                for ob in (2, 3):
                    if USE_DR:
                        tensor.matmul(
                            pp[ob][:, :], lhsT=wen_dr(ob, 0), rhs=esb_dr(0),
                            start=True, stop=False, perf_mode=DR,
                        )
                        mm = tensor.matmul(
                            pp[ob][:, :], lhsT=wen_dr(ob, 1), rhs=esb_dr(1),
                            start=False, stop=True, perf_mode=DR,
                        )
                    else:
                        for k in range(NB):
                            mm = tensor.matmul(
                                pp[ob][:, :], lhsT=wen_k(ob, k), rhs=esb_k(k),
                                start=(k == 0), stop=(k == NB - 1),
                            )
                    mm.then_inc(pe, 1)  # pe 7,8
                # C: S^T[jb] += ee[ob].T @ ed[ob], ob-major rounds so each
                # round starts as soon as its exp lands; two phases of two
                # concurrent groups (one per ps bank)
                for phase, jbs in enumerate(((0, 1), (2, 3))):
                    for ob in range(NB):
                        if phase == 0:
                            tensor.wait_ge(ac, (4, 4, 5, 6)[ob])
                        for jb in jbs:
                            bank, slot = C_DST[jb]
                            mm = tensor.matmul(
                                (psY if bank else psX)[:, slot, :],
                                lhsT=i8(ee[:, ob, jb * IB : (jb + 1) * IB]),
                                rhs=i8(ed[:, ob, :]),
                                start=(ob == 0),
                                stop=(ob == NB - 1),
                            )
                            if ob == NB - 1:
                                mm.then_inc(pe, 1)  # pe 9..12
                # E: pc[ob] += enc[jb].T @ LSE'^T[jb]
                tensor.wait_ge(dN, 32)
                tensor.wait_ge(ac, 8)
                for ob in range(NB):
                    for jb in (0, 2, 1, 3):
                        mm = tensor.matmul(
                            pcb[:, ob, :],
                            lhsT=i8(eNs[:, jb, ob * IB : (ob + 1) * IB]),
                            rhs=i8(lt[:, LT_SLOT[jb], :]),
                            start=(jb == 0),
                            stop=(jb == 3),
                        )
                        if jb == 3:
                            mm.then_inc(pe, 1)  # pe 13..16

            @block.vector
            def _(vector):
                if not PREAMBLE_MEMSET:
                    vector.memset(jbf[:, :], 0.0).then_inc(jz, 1)
                # S_enc = row-sum of enc^T
                vector.wait_ge(dE, 32)
                vector.reduce_sum(
                    out=se[:, :],
                    in_=esb[:, :, :].bitcast(FP8),
                    axis=mybir.AxisListType.X,
                ).then_inc(dv, 1)  # dv 1
                vector.wait_ge(dB, 16)
                # fix[ob] = sum_j (enc_part^T + (b-MU)) * enc^T, fused
                for ob in range(NB):
                    vector.wait_ge(pe, (1, 2, 7, 8)[ob])
                    vector.scalar_tensor_tensor(
                        out=i8(junkD[:, ob, :]) if FP8_INTER else junkD[:, ob, :],
                        in0=pp[ob][:, :],
                        scalar=b4m[:, ob : ob + 1],
                        in1=esb[:, ob, :].bitcast(FP8),
                        op0=ALU.add,
                        op1=ALU.mult,
                        accum_out=cp[:, ob : ob + 1],
                    ).then_inc(dv, 1)  # dv 2..5
                # ctmp = dec_part^T*se + fix  (pd complete since pe>=8 > 6)
                vector.wait_ge(dv, 5)  # own se + stts landed
                for ob in range(NB):
                    vector.tensor_scalar(
                        out=ctxo[:, ob, :],
                        in0=pd[:, ob, :],
                        scalar1=se[:, ob : ob + 1],
                        scalar2=cp[:, ob : ob + 1],
                        op0=ALU.mult,
                        op1=ALU.add,
                    ).then_inc(dv, 1)  # dv 6..9
                # ctx = ctmp - pc, in halves behind the E stops
                vector.wait_ge(pe, 14)
                vector.wait_ge(dv, 7)  # ctmp ob0/ob1 landed
                vector.tensor_tensor(
                    out=ctxo[:, 0:2, :],
                    in0=ctxo[:, 0:2, :],
                    in1=pcb[:, 0:2, :],
                    op=ALU.subtract,
                ).then_inc(dv, 1)  # dv 10
                vector.wait_ge(pe, 16)
                vector.wait_ge(dv, 9)  # ctmp ob2/ob3 landed
                vector.tensor_tensor(
                    out=ctxo[:, 2:NB, :],
                    in0=ctxo[:, 2:NB, :],
                    in1=pcb[:, 2:NB, :],
                    op=ALU.subtract,
                ).then_inc(dv, 1)  # dv 11

        nc.finalize()
    return nc


def _build_nc():
    return build_raw(bacc, bass=bass, mybir=mybir)


def _pack8(a):
    """fp8-quantize fp32 array and pack byte-pairs into bf16 slots
    (halves the last dim)."""
    a8 = np.ascontiguousarray(np.asarray(a, np.float32).astype(ml_dtypes.float8_e4m3))
    return a8.view(np.uint16).view(ml_dtypes.bfloat16)


def _prep_in_maps(encoderOutput, decoderInput, W, b):
    bf = ml_dtypes.bfloat16
    WT = np.ascontiguousarray(np.asarray(W, np.float32).T)  # [2H, H2]
    WdT = WT[:H2]  # [d, o]
    WeT = WT[H2:]  # [d, o]
    # wen[p, k, ob, :] = fp8(WeT[k*128+p, ob*128:(ob+1)*128])
    wen = _pack8(WeT.reshape(NB, P, NB, P).transpose(1, 0, 2, 3))
    wdec_flat = _pack8(WdT.reshape(NB, P, NB, P).transpose(1, 0, 2, 3)).reshape(
        P, 1024
    )
    b4 = np.asarray(b, np.float32).reshape(NB, P).T  # [P, 4]
    b4d = np.empty((P, 8), np.float32)
    b4d[:, 0:4] = b4
    b4d[:, 4:8] = b4 - MU
    in_maps = []
    for core in range(B):
        e = np.asarray(encoderOutput[core], np.float32)  # [512 j, 512 d]
        d = np.asarray(decoderInput[core], np.float32)  # [64 i, 512 d]
        # esb[p, k, j] = fp8(enc^T[k*128+p, j])
        esb = _pack8(e.T.reshape(NB, P, T_ENC).transpose(1, 0, 2))
        # dect[p, k, i] = fp8(dec^T[k*128+p, i])
        dect = _pack8(d.T.reshape(NB, P, T_DEC).transpose(1, 0, 2)).reshape(P, 128)
        bt = np.empty((P, 1152), bf)
        bt[:, :1024] = wdec_flat
        bt[:, 1024:] = dect
        # eN[p, jb, o] = enc[jb*128+p, o]
        eNt = e.reshape(NB, P, H2).transpose(1, 0, 2)
        if FP8_INTER:
            eN = _pack8(eNt)
        else:
            eN = np.ascontiguousarray(eNt).astype(bf).view(np.uint16).view(bf)
        in_maps.append({"esb": esb, "wen": wen, "b4": b4d, "bt": bt, "eN": eN})
    return in_maps


def _unshard_single(arr):
    # out[p, ob, i] = ctx^T[ob*128+p, i]  ->  ctx [T_dec, H2]
    a = np.asarray(arr, np.float32).reshape(P, NB, T_DEC)
    return a.transpose(1, 0, 2).reshape(H2, T_DEC).T


def kernel(encoderOutput, decoderInput, W, b, _trace=False):
    if "nc" not in _CACHE:
        _CACHE["nc"] = _build_nc()
    nc = _CACHE["nc"]
    in_maps = _prep_in_maps(encoderOutput, decoderInput, W, b)
    res = run_bass_kernel_spmd(nc, in_maps, core_ids=list(range(B)), trace=_trace)
    outs = np.stack([_unshard_single(r["out"]) for r in res.results])
    if _trace:
        _CACHE["last_result"] = res
    return outs


# revision 34
# speedup vs baseline: 1.0645x; 1.0645x over previous
"""Trainium2 Bass kernel for nn_Attention_28724741275862.

Reference computation (per batch b):
    dec_part[i,o] = dec[b] @ W_dec.T          # [64, 512]
    enc_part[j,o] = enc[b] @ W_enc.T          # [512, 512]
    logits[i,j,o] = dec_part[i,o] + enc_part[j,o] + bias[o]
    alpha = log_softmax(logits, axis=o)
    ctx[i,o] = sum_j alpha[i,j,o] * enc[b][j,o]

Factorization (exact in fp32; ~7e-3 max-rel with fp8 operands):
    LSE[i,j] = ln sum_o exp(dec_part[i,o]) * exp(enc_part[j,o] + b[o])
    LSE'     = LSE - MU   (MU hardcoded; realized as ln(S_mat * e^-MU))
    ctx[i,o] = dec_part[i,o]*S[o] + fix[o] - sum_j LSE'[i,j]*enc[j,o]
      S[o]   = sum_j enc[j,o]
      fix[o] = (b[o]-MU)*S[o] + sum_j enc_part[j,o]*enc[j,o]

Pipeline (per core = per batch; transposed layout: features on partitions):
    A1 (PE):  enc_part^T = W_enc^T.T @ enc^T, fp8 DoubleRow (2 d-blocks/pass)
    ee (ACT): exp(enc_part^T + b) per ob as each pp[ob] completes; fp8 out
    A2 (PE):  dec_part^T (fp8) slotted between A1 ob1 and ob2 so ed =
              exp(dec_part^T) lands before ee2/ee3 finish (ed gates all of C)
    C  (PE):  S_mat^T[j,i] += ee[ob].T @ ed[ob], per-ob rounds as exps land,
              two PSUM banks so two jb groups stay open concurrently
    Ln (ACT): LSE'^T = ln(S_mat^T * e^-MU) fp8, split per C bank; a single
              act-table set (6: ln+exp) is preloaded so there is NO mid-
              kernel Exp->Ln table switch (~2.4us saved)
    E  (PE):  pc[ob] += enc[j,o].T @ LSE'^T into the recycled pp0/pp1 banks
              (two banks => final subs start at pe>=14, not pe>=16)
    DVE:      S = reduce_sum(enc^T); fix via fused stts per ob (gated on the
              matching exp: ACT and DVE must never read the same PSUM bank
              concurrently -- that hangs the HW); ctmp = dec_part^T*S + fix;
              4 subs; out DMA in 2 halves.

DMA: full-128-partition transfers (64-partition halves run at half rate),
alternating HWDGE rings: SP carries wen -> b4 -> bt (+2 out halves),
ACT carries esb -> eNs.  All inputs fp8 => 1.06 MB/core (vs 1.64 MB).
PE HAM warmup junk matmuls run during the DMA phase.

Sharding: data-parallel over batch B=8 across 8 cores; no collectives.
Self-contained: hardcodes B=8, T_dec=64, T_enc=512, H2=512.
"""

import sys

for _p in ("/opt/trn_rl_repo",):
    if _p not in sys.path:
        sys.path.insert(0, _p)

import numpy as np
import ml_dtypes

import concourse.bass as bass
from concourse import bacc, mybir
from concourse.bass_utils import run_bass_kernel_spmd

B, T_DEC, T_ENC, H2 = 8, 64, 512, 512
P = 128  # SBUF partitions
NB = H2 // P  # 4 feature blocks
MU = 7.0  # LSE shift; ln(S*e^-MU) keeps LSE' in the fp8 sweet spot
N_WARMUP = 8  # PE HAM warmup matmuls (256-wide junk)
ACT_SET_LN_EXP = 6  # act-table set "natural_log_exp_and_others"

BF16 = mybir.dt.bfloat16
F32 = mybir.dt.float32

_CACHE = {}

from contextlib import ExitStack


def build_raw(bacc, mybir, bass):
    BF16 = mybir.dt.bfloat16
    F32 = mybir.dt.float32
    FP8 = mybir.dt.float8e4
    AF = mybir.ActivationFunctionType
    ALU = mybir.AluOpType
    DR = mybir.MatmulPerfMode.DoubleRow
    SCALE = float(np.exp(-MU))

    nc = bacc.Bacc(None, target_bir_lowering=False)

    # DRAM inputs, p-major: every DMA is one long contiguous run per partition.
    esbd = nc.dram_tensor("esb", [P, NB, 256], BF16, kind="ExternalInput")
    wend = nc.dram_tensor("wen", [P, NB, NB, 64], BF16, kind="ExternalInput")
    b4d = nc.dram_tensor("b4", [P, 8], F32, kind="ExternalInput")
    btd = nc.dram_tensor("bt", [P, 1152], BF16, kind="ExternalInput")
    eNd = nc.dram_tensor("eN", [P, NB, 256], BF16, kind="ExternalInput")
    outd = nc.dram_tensor("out", [P, NB, T_DEC], F32, kind="ExternalOutput")

    with ExitStack() as ctx:
        ec = ctx.enter_context
        # ---- SBUF ----
        esb = ec(nc.sbuf_tensor("esbs", [P, NB, 256], BF16))  # enc^T fp8
        wen = ec(nc.sbuf_tensor("wens", [P, NB, NB, 64], BF16))  # W_enc^T fp8
        b4s = ec(nc.sbuf_tensor("b4s", [P, 8], F32))  # b4 | b4m
        bt = ec(nc.sbuf_tensor("bts", [P, 1152], BF16))  # W_dec^T | dec^T fp8
        eNs = ec(nc.sbuf_tensor("eNs", [P, NB, 256], BF16))  # enc[j,o] fp8
        ee = ec(nc.sbuf_tensor("ee", [P, NB, 256], BF16))  # exp(enc_part^T+b)
        ed = ec(nc.sbuf_tensor("ed", [P, NB, 32], BF16))  # exp(dec_part^T)
        lt = ec(nc.sbuf_tensor("lt", [P, NB, 32], BF16))  # LSE'^T fp8
        se = ec(nc.sbuf_tensor("se", [P, NB], F32))  # S_enc
        cp = ec(nc.sbuf_tensor("cp", [P, NB], F32))  # fix
        junkD = ec(nc.sbuf_tensor("junkD", [P, NB, 512], BF16))  # stts sink
        ctxo = ec(nc.sbuf_tensor("ctxo", [P, NB, T_DEC], F32))
        jbf = ec(nc.sbuf_tensor("jbf", [P, 256], BF16))  # PE warmup junk
        wj = ec(nc.sbuf_tensor("wj", [P, NB], F32))  # ACT warmup junk
        # ---- PSUM (8 banks exactly) ----
        pp = [ec(nc.psum_tensor(f"pp{o}", [P, T_ENC], F32)) for o in range(NB)]
        pd = ec(nc.psum_tensor("pd", [P, NB, T_DEC], F32))
        # C output S^T: two banks so two jb accumulation groups can be open
        # at once (one group per PSUM zero-region). Slots: psX=(jb0,jb2),
        # psY=(jb1,jb3).
        psX = ec(nc.psum_tensor("psX", [P, 2, T_DEC], F32))
        psY = ec(nc.psum_tensor("psY", [P, 2, T_DEC], F32))
        pcb = ec(nc.psum_tensor("pcb", [P, NB, T_DEC], F32))  # spare

        # E outputs: ob0/ob1 -> recycled pp0 bank, ob2/ob3 -> pp1 bank
        # (both long dead by E time), so sub(ob0/1) runs while E still
        # writes the other bank.
        def e_dst(ob):
            base = (ob % 2) * 64
            return (pp[0] if ob < 2 else pp[1])[:, base : base + T_DEC]

        LT_SLOT = {0: 0, 2: 1, 1: 2, 3: 3}  # lt slot order (jb0, jb2, jb1, jb3)
        C_DST = {0: (0, 0), 1: (1, 0), 2: (0, 1), 3: (1, 1)}  # jb -> (bank, slot)

        b4ap = b4s[:, 0:4]  # [P, 4] true bias
        b4m = b4s[:, 4:8]  # [P, 4] bias - MU

        def wen_dr(ob, pair):  # A1 lhsT: [128 d, 2 kblk, 128 o] fp8
            return wen[:, 2 * pair : 2 * pair + 2, ob, :].bitcast(FP8)

        def esb_dr(pair):  # A1 rhs: [128 d, 2 kblk, 512 j] fp8
            return esb[:, 2 * pair : 2 * pair + 2, :].bitcast(FP8)

        def wdec_s(k, ob):  # A2 lhsT: [128 d, 128 o] fp8
            off = k * 256 + ob * 64
            return bt[:, off : off + 64].bitcast(FP8)

        def dect_s(k):  # A2 rhs: [128 d, 64 i] fp8
            off = 1024 + k * 32
            return bt[:, off : off + 32].bitcast(FP8)

        # ---- semaphores ----
        jz = ec(nc.semaphore("jz"))
        dE = ec(nc.semaphore("dE"))
        dE2 = ec(nc.semaphore("dE2"))
        dW = ec(nc.semaphore("dW"))
        dW2 = ec(nc.semaphore("dW2"))
        dB = ec(nc.semaphore("dB"))
        dA = ec(nc.semaphore("dA"))
        dN = ec(nc.semaphore("dN"))
        pe = ec(nc.semaphore("pe"))  # PE: A1ob0/1=1,2 A2=3-6 A1ob2/3=7,8 C=9-12 E=13-16
        ac = ec(nc.semaphore("ac"))  # ACT: warm=1 ee0=2 ee1=3 ed=4 ee2=5 ee3=6 ln=7,8
        dv = ec(nc.semaphore("dv"))  # DVE: se=1 stts=2-5 ctmp=6-9 subs=10-13
        dO = ec(nc.semaphore("dO"))  # out DMAs (2 x +16)

        def load_act_set(engine, set_id):
            inst = mybir.InstLoadActFuncSet(
                name=nc.get_next_instruction_name(),
                act_func_set_id=set_id,
                ins=[],
                outs=[],
            )
            return engine.add_instruction(inst)

        with nc.Block(no_gpsimd_drain=True) as block:

            @block.sync
            def _(sync):
                sync.dma_start(out=wen[:, 0:2], in_=wend[:, 0:2]).then_inc(dW, 16)
                sync.dma_start(out=wen[:, 2:NB], in_=wend[:, 2:NB]).then_inc(dW2, 16)
                sync.dma_start(out=b4s[:, :], in_=b4d[:, :]).then_inc(dB, 16)
                sync.dma_start(out=bt[:, :], in_=btd[:, :]).then_inc(dA, 16)
                sync.dma_start(out=eNs[:, :, :], in_=eNd[:, :, :]).then_inc(dN, 16)
                sync.wait_ge(dv, 11)
                sync.dma_start(out=outd[:, 0:2, :], in_=ctxo[:, 0:2, :]).then_inc(
                    dO, 16
                )
                sync.wait_ge(dv, 13)
                sync.dma_start(out=outd[:, 2:NB, :], in_=ctxo[:, 2:NB, :]).then_inc(
                    dO, 16
                )
                sync.wait_ge(dO, 32)

            @block.scalar
            def _(scalar):
                # Preload the one table set holding BOTH exp and ln; the
                # warm junk ops reference each func so the auto-insert pass
                # sees them covered and adds no further (blocking) loads.
                load_act_set(scalar, ACT_SET_LN_EXP)
                scalar.activation(wj[:, 0:1], wj[:, 3:4], AF.Exp, scale=0.0)
                scalar.activation(
                    wj[:, 1:2], wj[:, 3:4], AF.Ln, bias=1.0, scale=0.0
                ).then_inc(ac, 1)  # ac=1
                scalar.dma_start(out=esb[:, 0:2, :], in_=esbd[:, 0:2, :]).then_inc(
                    dE, 16
                )
                scalar.dma_start(out=esb[:, 2:NB, :], in_=esbd[:, 2:NB, :]).then_inc(
                    dE2, 16
                )
                scalar.wait_ge(dB, 16)
                # ee[ob] = exp(enc_part^T + b), fp8, as each pp[ob] completes
                for ob in range(NB):
                    scalar.wait_ge(pe, 1 + ob)
                    scalar.activation(
                        ee[:, ob, :].bitcast(FP8), pp[ob][:, :], AF.Exp,
                        bias=b4ap[:, ob : ob + 1],
                    ).then_inc(ac, 1)  # ac 2..5
                # ed = exp(dec_part^T); ACT is serial so this lands after ee3
                scalar.wait_ge(pe, 8)
                scalar.activation(
                    ed[:, :, :].bitcast(FP8), pd[:, :, :], AF.Exp
                ).then_inc(ac, 1)  # ac=6
                # LSE'^T = ln(S^T * e^-MU), fp8, split per C bank
                scalar.wait_ge(pe, 11)
                scalar.activation(
                    lt[:, 0:2, :].bitcast(FP8), psX[:, :, :], AF.Ln, scale=SCALE
                ).then_inc(ac, 1)  # ac=7
                scalar.wait_ge(pe, 12)
                scalar.activation(
                    lt[:, 2:NB, :].bitcast(FP8), psY[:, :, :], AF.Ln, scale=SCALE
                ).then_inc(ac, 1)  # ac=8

            @block.tensor
            def _(tensor):
                # HAM warmup junk matmuls into the (later overwritten) pd bank
                tensor.wait_ge(jz, 1)
                for _ in range(N_WARMUP):
                    tensor.matmul(
                        pd[:, :, :],
                        lhsT=jbf[:, 0:P],
                        rhs=jbf[:, :],
                        start=True,
                        stop=True,
                    )
                # A1 pair-major (fp8 DoubleRow): pair0 as soon as the k01
                # chunks land, pair1 behind the k23 chunks; stops staggered
                # by ob in the pair1 round
                tensor.wait_ge(dE, 16)
                tensor.wait_ge(dW, 16)
                for ob in range(NB):
                    tensor.matmul(
                        pp[ob][:, :], lhsT=wen_dr(ob, 0), rhs=esb_dr(0),
                        start=True, stop=False, perf_mode=DR,
                    )
                tensor.wait_ge(dE2, 16)
                tensor.wait_ge(dW2, 16)
                for ob in range(NB):
                    tensor.matmul(
                        pp[ob][:, :], lhsT=wen_dr(ob, 1), rhs=esb_dr(1),
                        start=False, stop=True, perf_mode=DR,
                    ).then_inc(pe, 1)  # pe 1..4
                # A2: dec_part^T (fp8)
                tensor.wait_ge(dA, 16)
                for ob in range(NB):
                    for k in range(NB):
                        mm = tensor.matmul(
                            pd[:, ob, :], lhsT=wdec_s(k, ob), rhs=dect_s(k),
                            start=(k == 0), stop=(k == NB - 1),
                        )
                        if k == NB - 1:
                            mm.then_inc(pe, 1)  # pe 5..8
                # C: S_mat^T, per-ob rounds as the exps land; two phases of
                # two concurrently-open jb groups (one per bank)
                tensor.wait_ge(ac, 6)  # all ee + ed
                for phase, jbs in enumerate(((0, 1), (2, 3))):
                    for ob in range(NB):
                        for jb in jbs:
                            bank, slot = C_DST[jb]
                            mm = tensor.matmul(
                                (psY if bank else psX)[:, slot, :],
                                lhsT=ee[:, ob, jb * 64 : (jb + 1) * 64].bitcast(FP8),
                                rhs=ed[:, ob, :].bitcast(FP8),
                                start=(ob == 0),
                                stop=(ob == NB - 1),
                            )
                            if ob == NB - 1:
                                mm.then_inc(pe, 1)  # pe 9..12
                # E: pc[ob] += enc[jb].T @ LSE'^T[jb]; ob groups sequential,
                # ob0/ob1 into pp0, ob2/ob3 into pp1
                tensor.wait_ge(dN, 16)
                tensor.wait_ge(ac, 8)
                tensor.wait_ge(dv, 3)  # stts0/1 done reading pp0/pp1
                for ob in range(NB):
                    for jb in (0, 2, 1, 3):
                        mm = tensor.matmul(
                            e_dst(ob),
                            lhsT=eNs[:, jb, ob * 64 : (ob + 1) * 64].bitcast(FP8),
                            rhs=lt[:, LT_SLOT[jb], :].bitcast(FP8),
                            start=(jb == 0),
                            stop=(jb == 3),
                        )
                        if jb == 3:
                            mm.then_inc(pe, 1)  # pe 13..16

            @block.vector
            def _(vector):
                vector.memset(jbf[:, :], 0.0).then_inc(jz, 1)
                # S_enc = row-sum of enc^T
                vector.wait_ge(dE, 16)
                vector.wait_ge(dE2, 16)
                vector.reduce_sum(
                    out=se[:, :],
                    in_=esb[:, :, :].bitcast(FP8),
                    axis=mybir.AxisListType.X,
                ).then_inc(dv, 1)  # dv 1
                vector.wait_ge(dB, 16)
                # fix[ob] = sum_j (enc_part^T + (b-MU)) * enc^T, fused stts.
                # Gated on the matching exp (ac): DVE must not read a PSUM
                # bank while ACT reads it -- concurrent reads hang the HW.
                for ob in range(NB):
                    vector.wait_ge(ac, 2 + ob)
                    vector.scalar_tensor_tensor(
                        out=junkD[:, ob, :],
                        in0=pp[ob][:, :],
                        scalar=b4m[:, ob : ob + 1],
                        in1=esb[:, ob, :].bitcast(FP8),
                        op0=ALU.add,
                        op1=ALU.mult,
                        accum_out=cp[:, ob : ob + 1],
                    ).then_inc(dv, 1)  # dv 2..5
                # ctmp = dec_part^T*se + fix (after ed is done with pd)
                vector.wait_ge(dv, 5)
                vector.wait_ge(ac, 6)
                for ob in range(NB):
                    vector.tensor_scalar(
                        out=ctxo[:, ob, :],
                        in0=pd[:, ob, :],
                        scalar1=se[:, ob : ob + 1],
                        scalar2=cp[:, ob : ob + 1],
                        op0=ALU.mult,
                        op1=ALU.add,
                    ).then_inc(dv, 1)  # dv 6..9
                # ctx = ctmp - pc, per ob behind the E stops
                vector.wait_ge(pe, 14)
                vector.wait_ge(dv, 7)
                for ob in (0, 1):
                    vector.tensor_tensor(
                        out=ctxo[:, ob, :],
                        in0=ctxo[:, ob, :],
                        in1=e_dst(ob),
                        op=ALU.subtract,
                    ).then_inc(dv, 1)  # dv 10,11
                vector.wait_ge(pe, 16)
                vector.wait_ge(dv, 9)
                for ob in (2, 3):
                    vector.tensor_tensor(
                        out=ctxo[:, ob, :],
                        in0=ctxo[:, ob, :],
                        in1=e_dst(ob),
                        op=ALU.subtract,
                    ).then_inc(dv, 1)  # dv 12,13

        nc.finalize()
    return nc


def _build_nc():
    return build_raw(bacc, bass=bass, mybir=mybir)


def _pack8(a):
    """fp8-quantize fp32 array and pack byte-pairs into bf16 slots
    (halves the last dim)."""
    a8 = np.ascontiguousarray(np.asarray(a, np.float32).astype(ml_dtypes.float8_e4m3))
    return a8.view(np.uint16).view(ml_dtypes.bfloat16)


def _prep_in_maps(encoderOutput, decoderInput, W, b):
    bf = ml_dtypes.bfloat16
    WT = np.ascontiguousarray(np.asarray(W, np.float32).T)  # [2H, H2]
    WdT = WT[:H2]  # [d, o]
    WeT = WT[H2:]  # [d, o]
    # wen[p, k, ob, :] = fp8(WeT[k*128+p, ob*128:(ob+1)*128])
    wen = _pack8(WeT.reshape(NB, P, NB, P).transpose(1, 0, 2, 3))
    wdec_flat = _pack8(WdT.reshape(NB, P, NB, P).transpose(1, 0, 2, 3)).reshape(
        P, 1024
    )
    b4 = np.asarray(b, np.float32).reshape(NB, P).T  # [P, 4]
    b4d = np.empty((P, 8), np.float32)
    b4d[:, 0:4] = b4
    b4d[:, 4:8] = b4 - MU
    in_maps = []
    for core in range(B):
        e = np.asarray(encoderOutput[core], np.float32)  # [512 j, 512 d]
        d = np.asarray(decoderInput[core], np.float32)  # [64 i, 512 d]
        # esb[p, k, j] = fp8(enc^T[k*128+p, j])
        esb = _pack8(e.T.reshape(NB, P, T_ENC).transpose(1, 0, 2))
        # dect[p, k, i] = fp8(dec^T[k*128+p, i])
        dect = _pack8(d.T.reshape(NB, P, T_DEC).transpose(1, 0, 2)).reshape(P, 128)
        bt = np.empty((P, 1152), bf)
        bt[:, :1024] = wdec_flat
        bt[:, 1024:] = dect
        # eN[p, jb, o] = fp8(enc[jb*128+p, o])
        eN = _pack8(e.reshape(NB, P, H2).transpose(1, 0, 2))
        in_maps.append({"esb": esb, "wen": wen, "b4": b4d, "bt": bt, "eN": eN})
    return in_maps


def _unshard_single(arr):
    # out[p, ob, i] = ctx^T[ob*128+p, i]  ->  ctx [T_dec, H2]
    a = np.asarray(arr, np.float32).reshape(P, NB, T_DEC)
    return a.transpose(1, 0, 2).reshape(H2, T_DEC).T


def kernel(encoderOutput, decoderInput, W, b, _trace=False):
    if "nc" not in _CACHE:
        _CACHE["nc"] = _build_nc()
    nc = _CACHE["nc"]
    in_maps = _prep_in_maps(encoderOutput, decoderInput, W, b)
    res = run_bass_kernel_spmd(nc, in_maps, core_ids=list(range(B)), trace=_trace)
    outs = np.stack([_unshard_single(r["out"]) for r in res.results])
    if _trace:
        _CACHE["last_result"] = res
    return outs
